# revision 15
# baseline (speedup 1.0000x reference)
"""DeepGESNCell kernel for 8 TRN2 NeuronCores.

h <- tanh(wiu + L @ (h @ W_hh^T)) iterated 10x, two layers, out = [h1|h2].

Strategy (row-sharded graph parallel):
  - core c owns rows I_c = [512c, 512c+512) of L; L[I_c].T stays resident in
    SBUF for all 20 iterations -> L is read from HBM exactly once.
  - Associativity: L @ (h @ W^T) == (L @ h) @ W^T.  Per iteration each core
    computes y_c^T = (L[I_c] @ h)^T with gathered-h tiles as the stationary
    operand and resident L^T tiles as the 512-wide moving operand, 2-way
    column-tiled across the PE array (two 64-wide matmuls share a 512-cycle
    slot -> full PE width).
  - z_c = y_c @ W_hh^T + wiu_c is produced in NORMAL layout by matmuls whose
    stationary operand is an SBUF copy of the stacked py halves (a stacked
    [W_hh^T; W_hh^T] moving operand folds the even/odd k halves' sum, an
    identity-matmul accumulates wiu), so tanh emits h_c [512, 64] directly.

Precision: double-bf16 split (L = L_hi + L_lo, h = h_hi + h_lo, keep the
three dominant terms, f32 PSUM accumulation) -> rel err ~3.6e-4.  Single
bf16/fp16 h fails or is too marginal (simulated 9.8e-2 / 1.4e-2 vs the
2e-2 gate), so the hi/lo pair of bf16 AllGathers per iteration stays.

Communication/scheduling (this revision):
  - cc buffers use a p-major layout ([128, 256] per rank block) so both the
    stage DMA (SBUF->cc_in) and the fetch DMAs (cc_out->SBUF) move 512B
    contiguous lines per partition; fetch is chunked by rank pairs and
    spread across the SP and Activation HWDGE queues.
  - The PE warm filler (keeps the PE clock from idling down during the
    gather hole) is right-sized: the old 36-matmul filler overshot the hole
    and, because the PE + LDWEIGHTS queues are in-order, delayed the first
    real matmul ~14us past data-ready every iteration.
"""

import os
import sys

import numpy as np

sys.path.insert(0, "/opt/trn_rl_repo")

import ml_dtypes

N = 4096
D_IN = 64
H = 64
MAX_ITER = 10
NCORES = 8
ROWS = N // NCORES  # 512
KT = N // 128  # 32 k-tiles over the node dim
RT = ROWS // 128  # 4 row-tiles per core chunk
WARM_N = int(os.environ.get("KERNEL_WARM_N", "0"))

_CACHE = {}
LAST_RESULTS = None


def _build_nc():
    import concourse.bacc as bacc
    import concourse.mybir as mybir
    import concourse.tile as tile
    from concourse import masks

    F32 = mybir.dt.float32
    BF = mybir.dt.bfloat16
    TANH = mybir.ActivationFunctionType.Tanh

    nc = bacc.Bacc(None, target_bir_lowering=False, num_devices=NCORES)

    LTH = nc.dram_tensor("LTH", [N, ROWS], BF, kind="ExternalInput")
    LTL = nc.dram_tensor("LTL", [N, ROWS], BF, kind="ExternalInput")
    XT = nc.dram_tensor("XT", [D_IN, ROWS], F32, kind="ExternalInput")
    WIH0 = nc.dram_tensor("WIH0T", [D_IN, H], F32, kind="ExternalInput")
    WST0 = nc.dram_tensor("WHH0TS", [2 * H, H], F32, kind="ExternalInput")
    WIH1 = nc.dram_tensor("WIH1T", [H, H], F32, kind="ExternalInput")
    WST1 = nc.dram_tensor("WHH1TS", [2 * H, H], F32, kind="ExternalInput")
    OUT = nc.dram_tensor("OUT", [ROWS, 2 * H], F32, kind="ExternalOutput")

    replica = [list(range(NCORES))]

    with tile.TileContext(nc) as tc:
        with (
            tc.tile_pool(name="cpool", bufs=1) as cpool,
            tc.tile_pool(name="spool", bufs=2) as spool,
            tc.tile_pool(name="ppool", bufs=2, space="PSUM") as ppool,
            tc.tile_pool(name="dpool", bufs=2, space="DRAM") as dpool,
        ):
            ident_bf = cpool.tile([128, 128], BF)
            masks.make_identity(nc, ident_bf[:])

            # small operands first so they don't queue behind the 8MB L load
            xt = cpool.tile([D_IN, ROWS], F32)
            nc.sync.dma_start(xt[:], XT.ap())
            wih0 = cpool.tile([D_IN, H], F32)
            nc.sync.dma_start(wih0[:], WIH0.ap())
            wst0 = cpool.tile([2 * H, H], F32)
            nc.sync.dma_start(wst0[:], WST0.ap())
            wih1 = cpool.tile([H, H], F32)
            nc.sync.dma_start(wih1[:], WIH1.ap())
            wst1 = cpool.tile([2 * H, H], F32)
            nc.sync.dma_start(wst1[:], WST1.ap())

            # resident L^T shard, hi/lo: lth[p, k, m] = bf16(L[I_c[m], 128k+p])
            # chunked + spread over both HWDGE queues; k=0 chunks land first
            # so the first iteration's passes can start early.
            lth = cpool.tile([128, KT, ROWS], BF)
            ltl = cpool.tile([128, KT, ROWS], BF)
            lth_src = LTH.ap().rearrange("(k p) m -> p k m", p=128)
            ltl_src = LTL.ap().rearrange("(k p) m -> p k m", p=128)
            for q in range(4):
                ks = slice(8 * q, 8 * (q + 1))
                nc.sync.dma_start(lth[:, ks, :], lth_src[:, ks, :])
                nc.scalar.dma_start(ltl[:, ks, :], ltl_src[:, ks, :])

            out_stage = cpool.tile([128, RT, 2 * H], F32)

            def warm_pe(n):
                # Filler matmuls that keep the PE's activity window busy in
                # the AllGather hole (>3.4us idle re-throttles the PE clock
                # to 1.2 GHz).  Sized to bridge the hole without jamming the
                # in-order PE queue ahead of the first data-dependent matmul.
                if n <= 0:
                    return
                pwarm = ppool.tile([64, ROWS], F32, tag="warm", name="pwarm", bufs=1)
                for _ in range(n):
                    nc.tensor.matmul(
                        pwarm[:, :],
                        ident_bf[:, 0:64],
                        lth[:, 0, :],
                        start=True,
                        stop=True,
                    )

            def gather_half(hs_hi, hs_lo, jlo, tagsuf):
                """AllGather the [hi|lo] bf16 pair for own-row half jlo.

                One fused 512KB collective per half so the half's gather can
                launch as soon as its two row-tiles are done (the other
                half's matmul still running), with both precisions of a
                rank block landing together.  cc layout is p-major: rank
                block = [128, 2, 2, H] -> 512B contiguous per partition."""
                cc_in = dpool.tile(
                    [128, 2, 2, H], BF, tag="cin" + tagsuf, name="cin" + tagsuf
                )
                nc.sync.dma_start(cc_in[:, 0], hs_hi[:, jlo : jlo + 2, :])
                nc.sync.dma_start(cc_in[:, 1], hs_lo[:, jlo : jlo + 2, :])
                cc_out = dpool.tile(
                    [NCORES, 128, 2 * 2 * H],
                    BF,
                    tag="cout" + tagsuf,
                    name="cout" + tagsuf,
                    addr_space="Shared",
                )
                nc.gpsimd.collective_compute(
                    "AllGather",
                    mybir.AluOpType.bypass,
                    replica_groups=replica,
                    ins=[cc_in.opt()],
                    outs=[cc_out.opt()],
                )
                return cc_out

            def fetch_half(cc_out, tagsuf):
                """DRAM [NCORES, 128, 256] -> SBUF, 4 rank-pair chunks.

                Each chunk is its OWN tile so the first matmuls' weight
                loads depend only on chunk 0; chunks are spread across both
                HWDGE queues; each partition line is 512B contiguous."""
                src = cc_out.rearrange("r p f -> p r f")
                chunks = []
                for q in range(4):
                    hq = spool.tile(
                        [128, 2, 2, 2, H],
                        BF,
                        tag=f"f{tagsuf}{q}",
                        name=f"f{tagsuf}{q}",
                    )
                    rs = slice(2 * q, 2 * q + 2)
                    eng = nc.sync if q % 2 == 0 else nc.scalar
                    eng.dma_start(
                        hq.rearrange("p r a j h -> p r (a j h)"), src[:, rs, :]
                    )
                    chunks.append(hq)
                return chunks

            def layer(wih, wst, xT_src, out_col):
                # wiu in normal layout (fp32): wiu_n[p, j, m] = wiu[I_c[128j+p], m]
                pw = ppool.tile([128, RT, H], F32, tag="pz", name="pw")
                for j in range(RT):
                    nc.tensor.matmul(
                        pw[:, j, :],
                        xT_src[:, 128 * j : 128 * (j + 1)],
                        wih[:],
                        start=True,
                        stop=True,
                    )
                # wiu split to bf16 hi/lo so the per-iteration wiu
                # accumulation matmuls are bf16 (hoistable before py without
                # leaving an fp32 accumulation group open across it) and run
                # as two F=256 matmuls instead of four fp32 F=64 ones.
                wiu_hi = spool.tile([128, RT, H], BF, tag="wiuh", name="wiu_hi")
                nc.vector.tensor_copy(wiu_hi[:], pw[:])
                wiu_lo = spool.tile([128, RT, H], BF, tag="wiul", name="wiu_lo")
                nc.vector.tensor_sub(wiu_lo[:], pw[:], wiu_hi[:])
                h_own = spool.tile([128, RT, H], F32, tag="h_own", name="h_own")
                nc.scalar.activation(h_own[:], pw[:], TANH)
                hs_hi = spool.tile([128, RT, H], BF, tag="hsh", name="hs_hi")
                nc.vector.tensor_copy(hs_hi[:], h_own[:])
                hs_lo = spool.tile([128, RT, H], BF, tag="hsl", name="hs_lo")
                nc.vector.tensor_sub(hs_lo[:], h_own[:], hs_hi[:])
                warm_pe(WARM_N)

                cc_a = gather_half(hs_hi, hs_lo, 0, "a")
                cc_b = gather_half(hs_hi, hs_lo, 2, "b")

                # k-tile pair order: A-half tiles (ranks' rows [0,256)) first
                # so the passes start on gather-a data while gather-b lands.
                ka = [(4 * r, 4 * r + 1) for r in range(NCORES)]
                kb = [(4 * r + 2, 4 * r + 3) for r in range(NCORES)]
                korder = ka + kb

                for _t in range(2, MAX_ITER + 1):
                    fa = fetch_half(cc_a, "a")
                    fb = fetch_half(cc_b, "b")

                    def at(k, prec):
                        r, t = k // 4, k % 4
                        ch = (fa if t < 2 else fb)[r // 2]
                        return ch[:, r % 2, prec, t % 2, :]

                    # pz opens with the wiu accumulation (bf16, before py so
                    # the post-py tail is just yab-copy + one matmul per j)
                    pz = ppool.tile([128, RT, H], F32, tag="pz", name="pz")
                    pzf = pz.rearrange("p t h -> p (t h)")
                    nc.tensor.matmul(
                        pzf[:, :], ident_bf[:], wiu_hi.rearrange("p t h -> p (t h)"),
                        start=True, stop=False, skip_group_check=True,
                    )
                    nc.tensor.matmul(
                        pzf[:, :], ident_bf[:], wiu_lo.rearrange("p t h -> p (t h)"),
                        start=False, stop=False, skip_group_check=True,
                    )
                    py = ppool.tile([128, ROWS], F32, tag="py", name="py")
                    # passes 1+2 (h_hi x L_hi, h_hi x L_lo), full 512 rows
                    for pi, ltx in enumerate((lth, ltl)):
                        for i, (k0, k1) in enumerate(korder):
                            st = pi == 0 and i == 0
                            nc.tensor.matmul(
                                py[0:64, :], at(k0, 0), ltx[:, k0, :],
                                start=st, stop=False,
                                tile_position=(0, 0), skip_group_check=True,
                            )
                            nc.tensor.matmul(
                                py[64:128, :], at(k1, 0), ltx[:, k1, :],
                                start=st, stop=False,
                                tile_position=(0, 64), skip_group_check=True,
                            )

                    h_own = spool.tile([128, RT, H], F32, tag="h_own", name="h_own")
                    hs_hi = spool.tile([128, RT, H], BF, tag="hsh", name="hs_hi")
                    hs_lo = spool.tile([128, RT, H], BF, tag="hsl", name="hs_lo")
                    yab = spool.tile([128, ROWS], F32, tag="yab", name="yab")
                    last = _t == MAX_ITER

                    # pass 3 (h_lo x L_hi) + tail, split by own-row halves:
                    # the half's rows finish, its y@W^T + tanh + split run
                    # and its fused gather launches while the other half's
                    # pass-3 matmuls still occupy the PE.
                    for half, (jlo, tagsuf) in enumerate(((0, "a"), (2, "b"))):
                        cols = slice(256 * half, 256 * (half + 1))
                        for i, (k0, k1) in enumerate(korder):
                            stp = i == len(korder) - 1
                            nc.tensor.matmul(
                                py[0:64, cols], at(k0, 1), lth[:, k0, cols],
                                start=False, stop=stp,
                                tile_position=(0, 0), skip_group_check=True,
                            )
                            nc.tensor.matmul(
                                py[64:128, cols], at(k1, 1), lth[:, k1, cols],
                                start=False, stop=stp,
                                tile_position=(0, 64), skip_group_check=True,
                            )
                        for j in (jlo, jlo + 1):
                            nc.vector.tensor_copy(
                                yab[:, 128 * j : 128 * (j + 1)],
                                py[:, 128 * j : 128 * (j + 1)],
                            )
                            nc.tensor.matmul(
                                pz[:, j, :],
                                yab[:, 128 * j : 128 * (j + 1)],
                                wst[:],
                                start=False,
                                stop=True,
                                skip_group_check=True,
                            )
                        for j in (jlo, jlo + 1):
                            nc.scalar.activation(h_own[:, j, :], pz[:, j, :], TANH)
                            nc.vector.tensor_copy(hs_hi[:, j, :], h_own[:, j, :])
                        for j in (jlo, jlo + 1):
                            nc.gpsimd.tensor_sub(
                                hs_lo[:, j, :], h_own[:, j, :], hs_hi[:, j, :]
                            )
                        if not last:
                            cc = gather_half(hs_hi, hs_lo, jlo, tagsuf)
                            if half == 0:
                                cc_a = cc
                            else:
                                cc_b = cc
                    warm_pe(WARM_N)

                nc.vector.tensor_copy(
                    out_stage[:, :, out_col : out_col + H], h_own[:]
                )
                return hs_hi, hs_lo

            h1_hi, h1_lo = layer(wih0, wst0, xt, 0)

            # boundary: h1^T [64, 512] fp32 for layer-1's wiu, via bf16
            # transposes of the hi/lo halves + f32 add (one PSUM operand max).
            ptr_hi = ppool.tile([64, ROWS], BF, tag="ptrh", name="ptr_hi", bufs=1)
            ptr_lo = ppool.tile([64, ROWS], BF, tag="ptrl", name="ptr_lo", bufs=1)
            for j in range(RT):
                nc.tensor.transpose(
                    ptr_hi[:, 128 * j : 128 * (j + 1)], h1_hi[:, j, :], ident_bf[:]
                )
                nc.tensor.transpose(
                    ptr_lo[:, 128 * j : 128 * (j + 1)], h1_lo[:, j, :], ident_bf[:]
                )
            h1T_hi = spool.tile([64, ROWS], F32, tag="h1Th", name="h1T_hi")
            nc.vector.tensor_copy(h1T_hi[:], ptr_hi[:])
            h1T = spool.tile([64, ROWS], F32, tag="h1T", name="h1T")
            nc.vector.tensor_add(h1T[:], h1T_hi[:], ptr_lo[:])

            layer(wih1, wst1, h1T, H)

            nc.sync.dma_start(
                OUT.ap().rearrange("(t p) h -> p t h", p=128), out_stage[:]
            )

    nc.compile()
    return nc


def _get_nc():
    if "nc" not in _CACHE:
        _CACHE["nc"] = _build_nc()
    return _CACHE["nc"]


def _ensure_ntff_hook():
    """bass_utils needs antenv.axon_hooks for trace=True under axon; the
    agent image's antenv lacks it.  Register an equivalent shim in
    sys.modules backed by ctypes calls into libaxon_pjrt.so."""
    import types

    try:
        import antenv.axon_hooks  # noqa: F401

        return
    except ImportError:
        pass
    mod = types.ModuleType("antenv.axon_hooks")
    state = {"hook": None, "tried": False}

    def set_axon_ntff_profile_hook(hook):
        state["hook"] = hook

    def get_axon_ntff_profile_hook():
        if state["hook"] is None and not state["tried"]:
            state["tried"] = True
            try:
                from trn_agent_boot.trn_boot import _ntff_profile_via_ctypes

                state["hook"] = _ntff_profile_via_ctypes(
                    "/opt/axon/libaxon_pjrt.so"
                )
            except Exception:
                state["hook"] = None
        return state["hook"]

    mod.set_axon_ntff_profile_hook = set_axon_ntff_profile_hook
    mod.get_axon_ntff_profile_hook = get_axon_ntff_profile_hook
    sys.modules["antenv.axon_hooks"] = mod


def kernel(X, L, W_ih0, W_hh0, W_ih1, W_hh1):
    global LAST_RESULTS
    _ensure_ntff_hook()
    from concourse.bass_utils import run_bass_kernel_spmd

    nc = _get_nc()
    f32 = np.float32
    bf = ml_dtypes.bfloat16

    wih0 = np.ascontiguousarray(np.asarray(W_ih0).T).astype(f32)
    wst0 = np.ascontiguousarray(
        np.concatenate([np.asarray(W_hh0).T, np.asarray(W_hh0).T], axis=0)
    ).astype(f32)
    wih1 = np.ascontiguousarray(np.asarray(W_ih1).T).astype(f32)
    wst1 = np.ascontiguousarray(
        np.concatenate([np.asarray(W_hh1).T, np.asarray(W_hh1).T], axis=0)
    ).astype(f32)

    Lf = np.asarray(L, dtype=f32)
    in_maps = []
    for c in range(NCORES):
        rows = slice(ROWS * c, ROWS * (c + 1))
        ltc = np.ascontiguousarray(Lf[rows, :].T)  # [N, ROWS] f32
        lth = ltc.astype(bf)
        ltl = (ltc - lth.astype(f32)).astype(bf)
        in_maps.append(
            {
                "LTH": lth,
                "LTL": ltl,
                "XT": np.ascontiguousarray(np.asarray(X)[rows, :].T).astype(f32),
                "WIH0T": wih0,
                "WHH0TS": wst0,
                "WIH1T": wih1,
                "WHH1TS": wst1,
            }
        )

    trace = bool(int(os.environ.get("KERNEL_TRACE", "0")))
    res = run_bass_kernel_spmd(
        nc, in_maps, core_ids=list(range(NCORES)), trace=trace
    )
    LAST_RESULTS = res
    out = np.concatenate([r["OUT"] for r in res.results], axis=0)
    return np.asarray(out, dtype=np.float32)
